# revision 13
# baseline (speedup 1.0000x reference)
"""Trainium2 Bass kernel for nn_BaseHashCode (prefix-hash of ragged sequences).

Reference computation (per row of `sequences` [B, 64], int32 digits 0..7):
    acc_t  = sum_{i<=t} a_i * x_i                      (int, < 2^29)
    pid_t  = ((acc_t + b) % 1000003) % 65536
    len    = #nonzero digits in the row
    out_t  = pid_t          if t < len
           = pid_{len-1}    otherwise   (len==0 -> pid_63, and then all pid equal)

Strategy: pure data parallel over 8 NeuronCores (batch shard).  Per core,
batch-major tiles [128 partitions x FD free] (FD/64 rows of 64 per partition).

No mod/divide exists in the DVE ISA, so the modulus is computed exactly in
fp32/int32 pieces:
  * a is split 8/12:  a = ahi*4096 + alo  (ahi < 2^8, alo < 2^12), so the two
    prefix sums S_hi <= 64*7*255+3 < 2^17 and S_lo <= 64*7*4095+57 < 2^21 stay
    exactly representable in fp32 (tensor_tensor_scan state is fp32).
  * b folds into the scan initial values (b = bhi*4096 + blo).
  * q = rne(acc_f/p) with acc_f = 4096*S_hi + S_lo (fp32, err<=32 -> |q-acc/p|
    < 0.5002), then r = acc - q*p is reconstructed EXACTLY via
    p = 244*4096 + 579:  rn = (244q - S_hi)*4096 + (579q - S_lo) = q*p - acc,
    every intermediate < 2^22.  r = (rn>0)*p - rn lands in [0, p).
  * pid = r & 0xffff  (bitwise AND is exact on int32 - HW-verified).
The ragged tail: len per row via (x!=0) + 3D tensor_reduce; C = pid[len-1]
via one-hot (iota+1 == max(len,1)) folded into a fused multiply+accumulate
(scalar_tensor_tensor accum_out); out = C + mask*(pid - C).
"""

import json

import numpy as np

import concourse.bass as bass
import concourse.mybir as mybir
from concourse.tile import TileContext
from concourse.bass_utils import run_bass_kernel_spmd


# ---------------------------------------------------------------------------
# BIR fixup: this container's walrus rejects instructions with too many
# sync_info.on_wait entries ("Too many sync wait commands").  Hoist excess
# waits onto injected same-engine NoOp instructions placed just before the
# offending instruction (same engine stream => identical semantics).  Only
# monotone waits (sem-ge-imm) are hoisted; eq-style waits stay put.
# ---------------------------------------------------------------------------
_WAIT_LIMIT = 1


def _fix_bir_sync_waits(bir_bytes: bytes, limit: int = _WAIT_LIMIT) -> bytes:
    bir = json.loads(bir_bytes)
    n_fixed = [0]

    def fix_list(insts):
        out = []
        for inst in insts:
            si = inst.get("sync_info") or {}
            ow = si.get("on_wait") or []
            if len(ow) > limit:
                movable = [w for w in ow if w.get("wait_mode") == "sem-ge-imm"]
                fixed = [w for w in ow if w.get("wait_mode") != "sem-ge-imm"]
                keep = (fixed + movable)[:limit]
                hoist = (fixed + movable)[limit:]
                if any(w.get("wait_mode") != "sem-ge-imm" for w in hoist):
                    out.append(inst)
                    continue
                for k in range(0, len(hoist), limit):
                    chunk = hoist[k : k + limit]
                    n_fixed[0] += 1
                    out.append(
                        {
                            "debug": inst.get("debug", 0),
                            "engine": inst["engine"],
                            "ins": [],
                            "name": f"{inst['name']}-wf{k}",
                            "opcode": "NoOp",
                            "outs": [],
                            "sync_info": {"on_wait": chunk},
                        }
                    )
                si = dict(si)
                si["on_wait"] = keep
                inst = dict(inst)
                inst["sync_info"] = si
            out.append(inst)
        return out

    def walk(o):
        if isinstance(o, dict):
            for k, v in o.items():
                if k == "instructions" and isinstance(v, list):
                    o[k] = fix_list(v)
                else:
                    walk(v)
        elif isinstance(o, list):
            for v in o:
                walk(v)

    walk(bir)
    if n_fixed[0]:
        return json.dumps(bir).encode()
    return bir_bytes


def _install_compile_patch():
    import concourse.bass_utils as bu
    import concourse.bass2jax as b2j

    if getattr(bu.compile_bir_kernel, "_waitfix", False):
        return
    orig = bu.compile_bir_kernel

    def patched(bir_json, tmpdir, neff_name="file.neff"):
        return orig(_fix_bir_sync_waits(bir_json), tmpdir, neff_name=neff_name)

    patched._waitfix = True
    bu.compile_bir_kernel = patched
    b2j.compile_bir_kernel = patched


_install_compile_patch()


PRIME = 1_000_003
P_HI = 244          # PRIME >> 12
P_LO = 579          # PRIME & 0xfff  (244*4096 + 579 == 1000003)
L = 64
N_CORES = 8
B_TOTAL = 1_048_576
ROWS_PER_CORE = B_TOTAL // N_CORES  # 131072

FD = 1024                    # free-dim elements per tile
RB = FD // L                 # rows per partition per tile
TILE_ROWS = 128 * RB
N_TILES = ROWS_PER_CORE // TILE_ROWS

AOT = mybir.AluOpType
F32 = mybir.dt.float32
I32 = mybir.dt.int32
COPY = mybir.ActivationFunctionType.Copy


def build_nc(b_val: int, rows: int = ROWS_PER_CORE, fd: int = FD):
    rb = fd // L
    tile_rows = 128 * rb
    n_tiles = rows // tile_rows
    assert rows % tile_rows == 0
    b_hi = float(int(b_val) >> 12)
    b_lo = float(int(b_val) & 0xFFF)

    nc = bass.Bass(target_bir_lowering=False)
    seq = nc.declare_dram_parameter("sequences", [rows, L], I32, isOutput=False)
    ahi_rep = nc.declare_dram_parameter("ahi_rep", [128, fd], F32, isOutput=False)
    alo_rep = nc.declare_dram_parameter("alo_rep", [128, fd], F32, isOutput=False)
    iotap1_rep = nc.declare_dram_parameter("iotap1_rep", [128, fd], F32, isOutput=False)
    out = nc.declare_dram_parameter("out", [rows, L], I32, isOutput=True)

    seq_t = seq.rearrange("(n p r) l -> n p (r l)", p=128, r=rb)
    out_t = out.rearrange("(n p r) l -> n p (r l)", p=128, r=rb)

    with TileContext(nc) as tc:
        with (
            tc.tile_pool(name="consts", bufs=1) as cpool,
            tc.tile_pool(name="work", bufs=2) as wpool,
        ):
            ahi_sb = cpool.tile([128, fd], F32, tag="ahi")
            alo_sb = cpool.tile([128, fd], F32, tag="alo")
            io_sb = cpool.tile([128, fd], F32, tag="io")
            nc.sync.dma_start(out=ahi_sb[:, :], in_=ahi_rep[:, :])
            nc.sync.dma_start(out=alo_sb[:, :], in_=alo_rep[:, :])
            nc.sync.dma_start(out=io_sb[:, :], in_=iotap1_rep[:, :])
            io3 = io_sb[:, :].rearrange("p (r l) -> p r l", l=L)

            for n in range(n_tiles):
                x_i = wpool.tile([128, fd], I32, tag="x")
                nc.sync.dma_start(out=x_i[:, :], in_=seq_t[n])

                x_f = wpool.tile([128, fd], F32, tag="xf")
                nc.scalar.activation(x_f[:, :], x_i[:, :], COPY)

                thi = wpool.tile([128, fd], F32, tag="thi")
                nc.vector.tensor_tensor(thi[:, :], x_f[:, :], ahi_sb[:, :], AOT.mult)
                tlo = wpool.tile([128, fd], F32, tag="tlo")
                nc.gpsimd.tensor_tensor(tlo[:, :], x_f[:, :], alo_sb[:, :], AOT.mult)

                shi = wpool.tile([128, fd], F32, tag="shi")
                slo = wpool.tile([128, fd], F32, tag="slo")
                for r in range(rb):
                    sl = slice(r * L, (r + 1) * L)
                    nc.vector.tensor_tensor_scan(
                        shi[:, sl], thi[:, sl], thi[:, sl], b_hi, AOT.add, AOT.bypass
                    )
                    nc.vector.tensor_tensor_scan(
                        slo[:, sl], tlo[:, sl], tlo[:, sl], b_lo, AOT.add, AOT.bypass
                    )

                # Barrett: q = rne(acc/p); r = q*p - acc reconstructed exactly
                accf = wpool.tile([128, fd], F32, tag="accf")
                nc.vector.scalar_tensor_tensor(
                    accf[:, :], shi[:, :], 4096.0, slo[:, :], AOT.mult, AOT.add
                )
                q = wpool.tile([128, fd], I32, tag="q")
                nc.vector.tensor_scalar(
                    q[:, :], accf[:, :], float(1.0 / PRIME), None, AOT.mult
                )
                u = wpool.tile([128, fd], F32, tag="u")
                nc.vector.scalar_tensor_tensor(
                    u[:, :], q[:, :], float(P_HI), shi[:, :], AOT.mult, AOT.subtract
                )
                v = wpool.tile([128, fd], F32, tag="v")
                nc.vector.scalar_tensor_tensor(
                    v[:, :], q[:, :], float(P_LO), slo[:, :], AOT.mult, AOT.subtract
                )
                rn = wpool.tile([128, fd], F32, tag="rn")
                nc.vector.scalar_tensor_tensor(
                    rn[:, :], u[:, :], 4096.0, v[:, :], AOT.mult, AOT.add
                )
                m = wpool.tile([128, fd], F32, tag="m")
                nc.gpsimd.tensor_scalar(m[:, :], rn[:, :], 0.0, None, AOT.is_gt)
                r_i = wpool.tile([128, fd], I32, tag="ri")
                nc.vector.scalar_tensor_tensor(
                    r_i[:, :], m[:, :], float(PRIME), rn[:, :], AOT.mult, AOT.subtract
                )
                pid = wpool.tile([128, fd], I32, tag="pid")
                nc.vector.tensor_scalar(
                    pid[:, :], r_i[:, :], 65535, None, AOT.bitwise_and
                )
                pid3 = pid[:, :].rearrange("p (r l) -> p r l", l=L)

                # ragged-tail bookkeeping
                w = wpool.tile([128, fd], F32, tag="w")
                nc.gpsimd.tensor_scalar(w[:, :], x_f[:, :], 0.5, None, AOT.is_gt)
                lens = wpool.tile([128, rb, 1], F32, tag="lens")
                nc.vector.tensor_reduce(
                    lens[:, :, :],
                    w[:, :].rearrange("p (r l) -> p r l", l=L),
                    mybir.AxisListType.X,
                    AOT.add,
                )
                lensc = wpool.tile([128, rb, 1], F32, tag="lensc")
                nc.vector.tensor_scalar(
                    lensc[:, :, :], lens[:, :, :], 1.0, None, AOT.max
                )
                mask = wpool.tile([128, fd], F32, tag="mask")
                mask3 = mask[:, :].rearrange("p (r l) -> p r l", l=L)
                nc.vector.tensor_tensor(
                    mask3, io3, lens[:, :, :].broadcast_to([128, rb, L]), AOT.is_le
                )
                oh = wpool.tile([128, fd], F32, tag="oh")
                oh3 = oh[:, :].rearrange("p (r l) -> p r l", l=L)
                nc.vector.tensor_tensor(
                    oh3, io3, lensc[:, :, :].broadcast_to([128, rb, L]), AOT.is_equal
                )

                # C[r] = pid[len-1] via fused one-hot dot per 64-block
                C = wpool.tile([128, rb], F32, tag="C")
                scr = wpool.tile([128, fd], F32, tag="scr")
                for r in range(rb):
                    sl = slice(r * L, (r + 1) * L)
                    nc.vector.scalar_tensor_tensor(
                        scr[:, sl], oh[:, sl], 1.0, pid[:, sl],
                        AOT.bypass, AOT.mult,
                        accum_out=C[:, r : r + 1],
                    )
                C3b = C[:, :].rearrange("p (r o) -> p r o", o=1).broadcast_to(
                    [128, rb, L]
                )

                # out = C + mask*(pid - C)
                d = wpool.tile([128, fd], F32, tag="d")
                d3 = d[:, :].rearrange("p (r l) -> p r l", l=L)
                nc.vector.tensor_tensor(d3, pid3, C3b, AOT.subtract)
                t2 = wpool.tile([128, fd], F32, tag="t2")
                nc.gpsimd.tensor_tensor(t2[:, :], mask[:, :], d[:, :], AOT.mult)
                o = wpool.tile([128, fd], I32, tag="o")
                o3 = o[:, :].rearrange("p (r l) -> p r l", l=L)
                nc.vector.tensor_tensor(o3, t2[:, :].rearrange("p (r l) -> p r l", l=L), C3b, AOT.add)

                nc.sync.dma_start(out=out_t[n], in_=o[:, :])

    return nc


_NC_CACHE: dict = {}


def _get_nc(b_val: int):
    key = (int(b_val), ROWS_PER_CORE, FD)
    if key not in _NC_CACHE:
        _NC_CACHE[key] = build_nc(int(b_val))
    return _NC_CACHE[key]


def make_const_inputs(a: np.ndarray, fd: int = FD):
    rb = fd // L
    a64 = a.astype(np.int64)
    ahi_rep = np.tile((a64 >> 12).astype(np.float32), (128, rb))
    alo_rep = np.tile((a64 & 0xFFF).astype(np.float32), (128, rb))
    iotap1_rep = np.tile(np.arange(1, L + 1, dtype=np.float32), (128, rb))
    return ahi_rep, alo_rep, iotap1_rep


def make_in_maps(sequences: np.ndarray, a: np.ndarray):
    ahi_rep, alo_rep, iotap1_rep = make_const_inputs(a)
    in_maps = []
    for i in range(N_CORES):
        shard = np.ascontiguousarray(
            sequences[i * ROWS_PER_CORE : (i + 1) * ROWS_PER_CORE].astype(
                np.int32, copy=False
            )
        )
        in_maps.append(
            {
                "sequences": shard,
                "ahi_rep": ahi_rep,
                "alo_rep": alo_rep,
                "iotap1_rep": iotap1_rep,
            }
        )
    return in_maps


def kernel(sequences: np.ndarray, a: np.ndarray, b) -> np.ndarray:
    sequences = np.asarray(sequences)
    a = np.asarray(a)
    assert sequences.shape == (B_TOTAL, L), sequences.shape

    nc = _get_nc(int(b))
    in_maps = make_in_maps(sequences, a)
    res = run_bass_kernel_spmd(nc, in_maps, core_ids=list(range(N_CORES)))
    outs = [res.results[i]["out"] for i in range(N_CORES)]
    return np.concatenate(outs, axis=0).astype(np.int32, copy=False)


if __name__ == "__main__":
    rng = np.random.default_rng(0)
    seqs = rng.integers(0, 8, size=(B_TOTAL, L), dtype=np.int32)
    a = rng.integers(1, PRIME, size=(L,), dtype=np.int32)
    out = kernel(sequences=seqs, a=a, b=12345)
    print(out.shape, out.dtype, out[:2, :8])


# revision 15
# speedup vs baseline: 1.3676x; 1.3676x over previous
"""Trainium2 Bass kernel for nn_BaseHashCode (prefix-hash of ragged sequences).

Reference computation (per row of `sequences` [B, 64], int32 digits 0..7):
    acc_t  = sum_{i<=t} a_i * x_i                      (int, < 2^29)
    pid_t  = ((acc_t + b) % 1000003) % 65536
    len    = #nonzero digits in the row
    out_t  = pid_t          if t < len
           = pid_{len-1}    otherwise   (len==0 -> pid_63, and then all pid equal)

Strategy: pure data parallel over 8 NeuronCores (batch shard).  Per core,
batch-major tiles [128 partitions x FD free] (FD/64 rows of 64 per partition).

No mod/divide exists in the DVE ISA, so the modulus is computed exactly in
fp32/int32 pieces:
  * a is split 8/12:  a = ahi*4096 + alo  (ahi < 2^8, alo < 2^12), so the two
    prefix sums S_hi <= 64*7*255+3 < 2^17 and S_lo <= 64*7*4095+57 < 2^21 stay
    exactly representable in fp32 (tensor_tensor_scan state is fp32).
  * b folds into the scan initial values (b = bhi*4096 + blo).
  * q = rne(acc_f/p) with acc_f = 4096*S_hi + S_lo (fp32, err<=32 -> |q-acc/p|
    < 0.5002), then r = acc - q*p is reconstructed EXACTLY via
    p = 244*4096 + 579:  rn = (244q - S_hi)*4096 + (579q - S_lo) = q*p - acc,
    every intermediate < 2^22.  r = (rn>0)*p - rn lands in [0, p).
  * pid = r & 0xffff  (bitwise AND is exact on int32 - HW-verified).
The ragged tail: len per row via (x!=0) + 3D tensor_reduce; C = pid[len-1]
via one-hot (iota+1 == max(len,1)) folded into a fused multiply+accumulate
(scalar_tensor_tensor accum_out); out = C + mask*(pid - C).
"""

import json

import numpy as np

import concourse.bass as bass
import concourse.mybir as mybir
from concourse.tile import TileContext
from concourse.bass_utils import run_bass_kernel_spmd


# ---------------------------------------------------------------------------
# BIR fixup: this container's walrus rejects instructions with too many
# sync_info.on_wait entries ("Too many sync wait commands").  Hoist excess
# waits onto injected same-engine NoOp instructions placed just before the
# offending instruction (same engine stream => identical semantics).  Only
# monotone waits (sem-ge-imm) are hoisted; eq-style waits stay put.
# ---------------------------------------------------------------------------
_WAIT_LIMIT = 1


def _fix_bir_sync_waits(bir_bytes: bytes, limit: int = _WAIT_LIMIT) -> bytes:
    bir = json.loads(bir_bytes)
    n_fixed = [0]

    def fix_list(insts):
        out = []
        for inst in insts:
            si = inst.get("sync_info") or {}
            ow = si.get("on_wait") or []
            if len(ow) > limit:
                movable = [w for w in ow if w.get("wait_mode") == "sem-ge-imm"]
                fixed = [w for w in ow if w.get("wait_mode") != "sem-ge-imm"]
                keep = (fixed + movable)[:limit]
                hoist = (fixed + movable)[limit:]
                if any(w.get("wait_mode") != "sem-ge-imm" for w in hoist):
                    out.append(inst)
                    continue
                for k in range(0, len(hoist), limit):
                    chunk = hoist[k : k + limit]
                    n_fixed[0] += 1
                    out.append(
                        {
                            "debug": inst.get("debug", 0),
                            "engine": inst["engine"],
                            "ins": [],
                            "name": f"{inst['name']}-wf{k}",
                            "opcode": "NoOp",
                            "outs": [],
                            "sync_info": {"on_wait": chunk},
                        }
                    )
                si = dict(si)
                si["on_wait"] = keep
                inst = dict(inst)
                inst["sync_info"] = si
            out.append(inst)
        return out

    def walk(o):
        if isinstance(o, dict):
            for k, v in o.items():
                if k == "instructions" and isinstance(v, list):
                    o[k] = fix_list(v)
                else:
                    walk(v)
        elif isinstance(o, list):
            for v in o:
                walk(v)

    walk(bir)
    if n_fixed[0]:
        return json.dumps(bir).encode()
    return bir_bytes


def _install_compile_patch():
    import concourse.bass_utils as bu
    import concourse.bass2jax as b2j

    if getattr(bu.compile_bir_kernel, "_waitfix", False):
        return
    orig = bu.compile_bir_kernel

    def patched(bir_json, tmpdir, neff_name="file.neff"):
        return orig(_fix_bir_sync_waits(bir_json), tmpdir, neff_name=neff_name)

    patched._waitfix = True
    bu.compile_bir_kernel = patched
    b2j.compile_bir_kernel = patched


_install_compile_patch()


PRIME = 1_000_003
P_HI = 244          # PRIME >> 12
P_LO = 579          # PRIME & 0xfff  (244*4096 + 579 == 1000003)
L = 64
N_CORES = 8
B_TOTAL = 1_048_576
ROWS_PER_CORE = B_TOTAL // N_CORES  # 131072

FD = 1024                    # free-dim elements per tile
RB = FD // L                 # rows per partition per tile
TILE_ROWS = 128 * RB
N_TILES = ROWS_PER_CORE // TILE_ROWS

AOT = mybir.AluOpType
F32 = mybir.dt.float32
I32 = mybir.dt.int32
COPY = mybir.ActivationFunctionType.Copy


def build_nc(b_val: int, rows: int = ROWS_PER_CORE, fd: int = FD):
    rb = fd // L
    tile_rows = 128 * rb
    n_tiles = rows // tile_rows
    assert rows % tile_rows == 0
    b_hi = float(int(b_val) >> 12)
    b_lo = float(int(b_val) & 0xFFF)

    nc = bass.Bass(target_bir_lowering=False)
    seq = nc.declare_dram_parameter("sequences", [rows, L], I32, isOutput=False)
    ahi_rep = nc.declare_dram_parameter("ahi_rep", [128, fd], F32, isOutput=False)
    alo_rep = nc.declare_dram_parameter("alo_rep", [128, fd], F32, isOutput=False)
    iotap1_rep = nc.declare_dram_parameter("iotap1_rep", [128, fd], F32, isOutput=False)
    out = nc.declare_dram_parameter("out", [rows, L], I32, isOutput=True)

    seq_t = seq.rearrange("(n p r) l -> n p (r l)", p=128, r=rb)
    out_t = out.rearrange("(n p r) l -> n p (r l)", p=128, r=rb)

    with TileContext(nc) as tc:
        with (
            tc.tile_pool(name="consts", bufs=1) as cpool,
            tc.tile_pool(name="work", bufs=2) as wpool,
        ):
            ahi_sb = cpool.tile([128, fd], F32, tag="ahi")
            alo_sb = cpool.tile([128, fd], F32, tag="alo")
            io_sb = cpool.tile([128, fd], F32, tag="io")
            nc.sync.dma_start(out=ahi_sb[:, :], in_=ahi_rep[:, :])
            nc.sync.dma_start(out=alo_sb[:, :], in_=alo_rep[:, :])
            nc.sync.dma_start(out=io_sb[:, :], in_=iotap1_rep[:, :])
            io3 = io_sb[:, :].rearrange("p (r l) -> p r l", l=L)

            for n in range(n_tiles):
                x_i = wpool.tile([128, fd], I32, tag="x")
                nc.sync.dma_start(out=x_i[:, :], in_=seq_t[n])

                x_f = wpool.tile([128, fd], F32, tag="xf")
                nc.scalar.activation(x_f[:, :], x_i[:, :], COPY)

                thi = wpool.tile([128, fd], F32, tag="thi")
                nc.vector.tensor_tensor(thi[:, :], x_f[:, :], ahi_sb[:, :], AOT.mult)
                tlo = wpool.tile([128, fd], F32, tag="tlo")
                nc.gpsimd.tensor_tensor(tlo[:, :], x_f[:, :], alo_sb[:, :], AOT.mult)

                shi = wpool.tile([128, fd], F32, tag="shi")
                slo = wpool.tile([128, fd], F32, tag="slo")
                for r in range(rb):
                    sl = slice(r * L, (r + 1) * L)
                    nc.vector.tensor_tensor_scan(
                        shi[:, sl], thi[:, sl], thi[:, sl], b_hi, AOT.add, AOT.bypass
                    )
                    nc.vector.tensor_tensor_scan(
                        slo[:, sl], tlo[:, sl], tlo[:, sl], b_lo, AOT.add, AOT.bypass
                    )

                # Barrett: q = rne(acc/p); r = q*p - acc reconstructed exactly
                accf = wpool.tile([128, fd], F32, tag="accf")
                nc.vector.scalar_tensor_tensor(
                    accf[:, :], shi[:, :], 4096.0, slo[:, :], AOT.mult, AOT.add
                )
                q = wpool.tile([128, fd], I32, tag="q")
                nc.vector.tensor_scalar(
                    q[:, :], accf[:, :], float(1.0 / PRIME), None, AOT.mult
                )
                u = wpool.tile([128, fd], F32, tag="u")
                nc.vector.scalar_tensor_tensor(
                    u[:, :], q[:, :], float(P_HI), shi[:, :], AOT.mult, AOT.subtract
                )
                v = wpool.tile([128, fd], F32, tag="v")
                nc.vector.scalar_tensor_tensor(
                    v[:, :], q[:, :], float(P_LO), slo[:, :], AOT.mult, AOT.subtract
                )
                rn = wpool.tile([128, fd], F32, tag="rn")
                nc.vector.scalar_tensor_tensor(
                    rn[:, :], u[:, :], 4096.0, v[:, :], AOT.mult, AOT.add
                )
                m = wpool.tile([128, fd], F32, tag="m")
                nc.vector.tensor_scalar(m[:, :], rn[:, :], 0.0, None, AOT.is_gt)
                r_i = wpool.tile([128, fd], I32, tag="ri")
                nc.vector.scalar_tensor_tensor(
                    r_i[:, :], m[:, :], float(PRIME), rn[:, :], AOT.mult, AOT.subtract
                )
                pid = wpool.tile([128, fd], I32, tag="pid")
                nc.vector.tensor_scalar(
                    pid[:, :], r_i[:, :], 65535, None, AOT.bitwise_and
                )
                pid3 = pid[:, :].rearrange("p (r l) -> p r l", l=L)

                # ragged-tail bookkeeping
                w = wpool.tile([128, fd], F32, tag="w")
                nc.gpsimd.tensor_scalar(w[:, :], x_f[:, :], 0.5, None, AOT.is_gt)
                lens = wpool.tile([128, rb, 1], F32, tag="lens")
                nc.vector.tensor_reduce(
                    lens[:, :, :],
                    w[:, :].rearrange("p (r l) -> p r l", l=L),
                    mybir.AxisListType.X,
                    AOT.add,
                )
                lensc = wpool.tile([128, rb, 1], F32, tag="lensc")
                nc.vector.tensor_scalar(
                    lensc[:, :, :], lens[:, :, :], 1.0, None, AOT.max
                )
                mask = wpool.tile([128, fd], F32, tag="mask")
                mask3 = mask[:, :].rearrange("p (r l) -> p r l", l=L)
                nc.vector.tensor_tensor(
                    mask3, io3, lens[:, :, :].broadcast_to([128, rb, L]), AOT.is_le
                )
                oh = wpool.tile([128, fd], F32, tag="oh")
                oh3 = oh[:, :].rearrange("p (r l) -> p r l", l=L)
                nc.vector.tensor_tensor(
                    oh3, io3, lensc[:, :, :].broadcast_to([128, rb, L]), AOT.is_equal
                )

                # C[r] = pid[len-1] via fused one-hot dot per 64-block
                C = wpool.tile([128, rb], F32, tag="C")
                scr = wpool.tile([128, fd], F32, tag="scr")
                for r in range(rb):
                    sl = slice(r * L, (r + 1) * L)
                    nc.vector.scalar_tensor_tensor(
                        scr[:, sl], oh[:, sl], 1.0, pid[:, sl],
                        AOT.bypass, AOT.mult,
                        accum_out=C[:, r : r + 1],
                    )
                C3b = C[:, :].rearrange("p (r o) -> p r o", o=1).broadcast_to(
                    [128, rb, L]
                )

                # out = C + mask*(pid - C)
                d = wpool.tile([128, fd], F32, tag="d")
                d3 = d[:, :].rearrange("p (r l) -> p r l", l=L)
                nc.vector.tensor_tensor(d3, pid3, C3b, AOT.subtract)
                t2 = wpool.tile([128, fd], F32, tag="t2")
                nc.vector.tensor_tensor(t2[:, :], mask[:, :], d[:, :], AOT.mult)
                o = wpool.tile([128, fd], I32, tag="o")
                o3 = o[:, :].rearrange("p (r l) -> p r l", l=L)
                nc.vector.tensor_tensor(o3, t2[:, :].rearrange("p (r l) -> p r l", l=L), C3b, AOT.add)

                nc.sync.dma_start(out=out_t[n], in_=o[:, :])

    return nc


_NC_CACHE: dict = {}


def _get_nc(b_val: int):
    key = (int(b_val), ROWS_PER_CORE, FD)
    if key not in _NC_CACHE:
        _NC_CACHE[key] = build_nc(int(b_val))
    return _NC_CACHE[key]


def make_const_inputs(a: np.ndarray, fd: int = FD):
    rb = fd // L
    a64 = a.astype(np.int64)
    ahi_rep = np.tile((a64 >> 12).astype(np.float32), (128, rb))
    alo_rep = np.tile((a64 & 0xFFF).astype(np.float32), (128, rb))
    iotap1_rep = np.tile(np.arange(1, L + 1, dtype=np.float32), (128, rb))
    return ahi_rep, alo_rep, iotap1_rep


def make_in_maps(sequences: np.ndarray, a: np.ndarray):
    ahi_rep, alo_rep, iotap1_rep = make_const_inputs(a)
    in_maps = []
    for i in range(N_CORES):
        shard = np.ascontiguousarray(
            sequences[i * ROWS_PER_CORE : (i + 1) * ROWS_PER_CORE].astype(
                np.int32, copy=False
            )
        )
        in_maps.append(
            {
                "sequences": shard,
                "ahi_rep": ahi_rep,
                "alo_rep": alo_rep,
                "iotap1_rep": iotap1_rep,
            }
        )
    return in_maps


def kernel(sequences: np.ndarray, a: np.ndarray, b) -> np.ndarray:
    sequences = np.asarray(sequences)
    a = np.asarray(a)
    assert sequences.shape == (B_TOTAL, L), sequences.shape

    nc = _get_nc(int(b))
    in_maps = make_in_maps(sequences, a)
    res = run_bass_kernel_spmd(nc, in_maps, core_ids=list(range(N_CORES)))
    outs = [res.results[i]["out"] for i in range(N_CORES)]
    return np.concatenate(outs, axis=0).astype(np.int32, copy=False)


if __name__ == "__main__":
    rng = np.random.default_rng(0)
    seqs = rng.integers(0, 8, size=(B_TOTAL, L), dtype=np.int32)
    a = rng.integers(1, PRIME, size=(L,), dtype=np.int32)
    out = kernel(sequences=seqs, a=a, b=12345)
    print(out.shape, out.dtype, out[:2, :8])
